# revision 1
# baseline (speedup 1.0000x reference)
"""Causal GQA self-attention (dense_transformer) on 8 trn2 NeuronCores.

Sharding: core c -> (batch b = c//4, kv-group g = c%4).  Each core computes
the 4 query heads of its kv group against its batch element, plus the
partial output projection for those heads; the host sums the 4 partial
projections per batch (the all-reduce of the tensor-parallel proj).

On-device layout is fully transposed ("feature on partitions"):
  xT [D, S], qhat/khat [head_dim, S], scoresT [k_pos, q_pos], yT [head_dim, S],
  outT [D_out, S].  Softmax runs without max subtraction: q/k are
  RMS-normalized so |scores| <= gain*sqrt(head_dim); exp cannot overflow.

v4 structure (single in-order PE queue, so program order == PE order):
  warmup | A(s0) | A(s1)c | B(0)attn | A(s1)chK | B(0)prj:0-9 | A(s1)chQ
         | A(s2)c | B(1)attn+B(0)prj:10-15 | A(s2)chK | B(1)prj:0-9 | A(s2)chQ
         | A(s3)c | B(2)attn+B(1)prj:10-15 | A(s3)chK | B(2)prj:0-9 | A(s3)chQ
         | B(3)attn+B(2)prj:10-15 | B(3)prj
  B segments fill the PE while the A-drain chains run on ACT/DVE/GPSIMD;
  proj matmuls spread into the next attention's head gaps give the ACT
  engine room to drain its exp backlog (exp is the pace-setter in pure
  attention stretches).  PSUM: 8 banks shared via pool tags qp0-3/kp/vp
  (+2 misc) with WAR tracking.  DMA: startup weight quads + x on the SP
  HWDGE queue (4KB descriptors), bulk weights/tables/outputs on the ACT
  HWDGE queue.
"""

import numpy as np

import concourse.bacc as bacc
import concourse.mybir as mybir
import concourse.tile as tile
from concourse.bass_utils import run_bass_kernel_spmd

F32 = mybir.dt.float32
F32R = mybir.dt.float32r
BF16 = mybir.dt.bfloat16
import os as _os


def _dt_env(name, default="bf16"):
    return BF16 if _os.environ.get(name, default) == "bf16" else F32R


QKV_DT = _dt_env("KERNEL_QKV_DT")
SC_DT = _dt_env("KERNEL_SC_DT")
AV_DT = _dt_env("KERNEL_AV_DT")
PR_DT = _dt_env("KERNEL_PR_DT")
AF = mybir.ActivationFunctionType
ALU = mybir.AluOpType

B = 2
S = 2048
D = 2048
N_HEADS = 16
N_KV = 4
HD = 128
G = N_HEADS // N_KV
ROPE_BASE = 10000.0
RMS_EPS = 1.1920928955078125e-07
NCH = D // 128
ST = 512
QT = 512
KC = 128


def _build_program(s_len=S):
    assert QKV_DT == SC_DT == AV_DT == BF16, "packed tables assume bf16"
    nst = s_len // ST
    nqt = s_len // QT
    nc = bacc.Bacc("TRN2", target_bir_lowering=False, debug=False, num_devices=8)

    # quad-major layouts: each DMA line is >=4KB per partition
    xa = nc.dram_tensor("xa", [nst, 4, 128, 4, ST], QKV_DT, kind="ExternalInput")
    wq = nc.dram_tensor("wq", [4, 128, 4, G * HD], QKV_DT, kind="ExternalInput")
    wk = nc.dram_tensor("wk", [4, 128, 4, HD], QKV_DT, kind="ExternalInput")
    wv = nc.dram_tensor("wv", [4, 128, 4, HD], QKV_DT, kind="ExternalInput")
    wp = nc.dram_tensor("wp", [128, G, D], PR_DT, kind="ExternalInput")
    cos2 = nc.dram_tensor("cos2", [128, s_len], SC_DT, kind="ExternalInput")
    sin2 = nc.dram_tensor("sin2", [128, s_len], SC_DT, kind="ExternalInput")
    gains = nc.dram_tensor("gains", [128, G], F32, kind="ExternalInput")
    # tabs: swp | idn | ons | msk packed on the free dim
    tabs = nc.dram_tensor("tabs", [128, 4, 128], BF16, kind="ExternalInput")
    # out[p, qt, dc, q~] = proj_partial[128*dc + p, 512*qt + q~]
    out = nc.dram_tensor("out", [128, nqt, D // 128, QT], BF16,
                         kind="ExternalOutput")

    with tile.TileContext(nc) as tc:
        with tc.tile_pool(name="persist", bufs=1) as pp, \
             tc.tile_pool(name="const", bufs=1) as cp, \
             tc.tile_pool(name="wA", bufs=1) as wa, \
             tc.tile_pool(name="xs", bufs=2) as xp, \
             tc.tile_pool(name="sbA", bufs=2) as sa, \
             tc.tile_pool(name="sbB", bufs=3) as sb, \
             tc.tile_pool(name="sbC", bufs=2) as sc_pool, \
             tc.tile_pool(name="psA", bufs=1, space="PSUM") as psa, \
             tc.tile_pool(name="psAm", bufs=2, space="PSUM") as psm:
            qhat = pp.tile([128, G, s_len], SC_DT)
            khat = pp.tile([128, s_len], SC_DT)
            vnat = pp.tile([128, s_len // 128, HD], AV_DT)
            yn = pp.tile([128, G, s_len], PR_DT)

            tabs_sb = cp.tile([128, 4, 128], BF16, tag="tabs")
            swp_sb = tabs_sb[:, 0, :]
            idn_sb = tabs_sb[:, 1, :]
            ons_sb = tabs_sb[:, 2, :]
            msk_sb = tabs_sb[:, 3, :]
            gains_sb = cp.tile([128, G], F32, tag="gains")
            onsr_sb = cp.tile([128, 128], QKV_DT, tag="onsr")
            nc.gpsimd.memset(onsr_sb[:], 1.0)
            epsq_sb = cp.tile([128, 1], F32, tag="epsq")
            epsk_sb = cp.tile([128, 1], F32, tag="epsk")
            nc.gpsimd.memset(epsq_sb[:], RMS_EPS)
            nc.gpsimd.memset(epsk_sb[:], HD * RMS_EPS)
            warm_sb = cp.tile([128, 1], F32, tag="warm")
            nc.scalar.activation(warm_sb[:], epsq_sb[:], AF.Square)
            nc.scalar.activation(warm_sb[:], epsq_sb[:], AF.Sqrt)
            nc.scalar.activation(warm_sb[:], epsq_sb[:], AF.Exp)
            wp_sb = cp.tile([128, G, D], PR_DT, tag="wp")

            wq_sb = wa.tile([128, NCH, G * HD], QKV_DT, tag="wq")
            wk_sb = wa.tile([128, NCH, HD], QKV_DT, tag="wk")
            wv_sb = wa.tile([128, NCH, HD], QKV_DT, tag="wv")
            cos_sb = wa.tile([128, s_len], SC_DT, tag="cos")
            sin_sb = wa.tile([128, s_len], SC_DT, tag="sin")

            # startup: wq + x interleaved on the fast SP queue (first c-loop
            # consumes quad g at start + 5.2us*g); wk/wv/tables on ACT queue
            nc.sync.dma_start(out=wq_sb[:, 0:4, :], in_=wq[0])
            nc.scalar.dma_start(out=wk_sb[:, 0:4, :], in_=wk[0])
            nc.scalar.dma_start(out=wv_sb[:, 0:4, :], in_=wv[0])
            for g in range(1, 4):
                nc.scalar.dma_start(out=wk_sb[:, 4 * g:4 * g + 4, :], in_=wk[g])
                nc.scalar.dma_start(out=wv_sb[:, 4 * g:4 * g + 4, :], in_=wv[g])
            nc.scalar.dma_start(out=tabs_sb[:], in_=tabs[:])
            nc.scalar.dma_start(out=gains_sb[:], in_=gains[:])
            nc.scalar.dma_start(out=cos_sb[:, 0:ST], in_=cos2[:, 0:ST])
            nc.scalar.dma_start(out=sin_sb[:, 0:ST], in_=sin2[:, 0:ST])

            # PE warmup: junk matmuls (no DMA dependency) keep the PE busy
            # through the startup DMA so the HAM clock gate is at 2.4GHz
            # when the first projection matmul issues.
            wu = cp.tile([128, 256], QKV_DT, tag="wu")
            nc.gpsimd.memset(wu[:], 0.0)
            wu_ps = psm.tile([128, ST], F32, tag="misc", name="wups")
            for _ in range(30):
                nc.tensor.matmul(wu_ps[:, 0:256], wu[:, 0:128], wu[:],
                                 start=True, stop=True)

            # ------------- emit helpers -------------
            def a_cloop(st):
                xs_t = xp.tile([128, NCH, ST], QKV_DT, tag="xs")
                for g in range(4):
                    nc.sync.dma_start(out=xs_t[:, 4 * g:4 * g + 4, :],
                                      in_=xa[st, g])
                    if st == 0 and g < 3:
                        nc.sync.dma_start(out=wq_sb[:, 4 * (g + 1):4 * (g + 1) + 4, :],
                                          in_=wq[g + 1])
                if st == 1:
                    nc.scalar.dma_start(out=cos_sb[:, ST:s_len], in_=cos2[:, ST:s_len])
                    nc.scalar.dma_start(out=sin_sb[:, ST:s_len], in_=sin2[:, ST:s_len])
                    nc.scalar.dma_start(out=wp_sb[:], in_=wp[:])
                qp = [psa.tile([128, ST], F32, tag=f"qp{h}", name=f"qp{h}") for h in range(G)]
                kp = psa.tile([128, ST], F32, tag="kp")
                vp = psa.tile([128, ST], F32, tag="vp")
                for c in range(NCH):
                    for h in range(G):
                        nc.tensor.matmul(qp[h][:], wq_sb[:, c, h * HD:(h + 1) * HD],
                                         xs_t[:, c, :], start=(c == 0), stop=(c == NCH - 1))
                    nc.tensor.matmul(kp[:], wk_sb[:, c, :], xs_t[:, c, :],
                                     start=(c == 0), stop=(c == NCH - 1))
                    nc.tensor.matmul(vp[:], wv_sb[:, c, :], xs_t[:, c, :],
                                     start=(c == 0), stop=(c == NCH - 1))
                return qp, kp, vp

            def a_drain(st, qp, kp, vp):
                """DVE-only psum drains, ordered by what the following b_attn
                unblocks first.  Emitted BEFORE b_attn so the shared psum
                banks hand off without a cross-engine dependency cycle."""
                raws = {}
                for t in [G, 0, 1, "v", 2, 3]:
                    if t == "v":
                        raw = sa.tile([128, ST], QKV_DT, tag="vraw")
                        nc.vector.tensor_copy(raw[:], vp[:])
                    else:
                        src = qp[t] if t < G else kp
                        raw = sa.tile([128, ST], SC_DT, tag="raw", name=f"raw{t}", bufs=6)
                        nc.vector.tensor_copy(raw[:], src[:])
                    raws[t] = raw
                return raws

            def _chain_one(st, t, raw, rope_dve=False):
                s0 = st * ST
                is_q = t < G
                sq = sa.tile([128, ST], QKV_DT, tag="sq", name=f"sq{t}", bufs=2)
                nc.scalar.activation(sq[:], raw[:], AF.Square)
                smq = psm.tile([128, ST], F32, tag="misc", name="smq")
                nc.tensor.matmul(smq[:], onsr_sb[:], sq[:], start=True, stop=True)
                den = sa.tile([128, ST], F32, tag="den")
                if is_q:
                    nc.scalar.activation(den[:], smq[:], AF.Sqrt,
                                         scale=1.0 / HD, bias=epsq_sb[:, 0:1])
                else:
                    nc.scalar.activation(den[:], smq[:], AF.Sqrt,
                                         scale=1.0, bias=epsk_sb[:, 0:1])
                rcp = sa.tile([128, ST], F32, tag="rcp")
                nc.vector.reciprocal_approx_fast(rcp[:], den[:])
                # q chains park the rope-swap psum on the (idle) qp banks so
                # the 2-deep misc ring doesn't serialize the chains at
                # DVE-chain latency
                qsw = psm.tile([128, ST], F32, tag="misc", name="qsw") if not is_q \
                    else psa.tile([128, ST], F32, tag=f"qp{t}", name="qsw")
                nc.tensor.matmul(qsw[:], swp_sb, raw[:], start=True, stop=True)
                m1 = sa.tile([128, ST], SC_DT, tag="m1")
                nc.vector.tensor_mul(m1[:], raw[:], cos_sb[:, s0:s0 + ST])
                m2 = sa.tile([128, ST], SC_DT, tag="m2")
                nc.vector.tensor_mul(m2[:], qsw[:], sin_sb[:, s0:s0 + ST])
                rope = sa.tile([128, ST], SC_DT, tag="rope")
                if rope_dve:
                    nc.vector.tensor_add(rope[:], m1[:], m2[:])
                else:
                    nc.gpsimd.tensor_add(rope[:], m1[:], m2[:])
                if is_q:
                    nc.vector.scalar_tensor_tensor(
                        out=qhat[:, t, s0:s0 + ST], in0=rope[:],
                        scalar=gains_sb[:, t:t + 1], in1=rcp[:],
                        op0=ALU.mult, op1=ALU.mult)
                else:
                    nc.gpsimd.tensor_mul(khat[:, s0:s0 + ST], rope[:], rcp[:])

            def a_chains_k(st, raws):
                _chain_one(st, G, raws[G])

            def a_chains_q(st, raws, rope_dve=False):
                for t in range(G):
                    _chain_one(st, t, raws[t], rope_dve=rope_dve)
                # v transposes last: their misc-ring WARs resolve during the
                # preceding proj, and vnat isn't needed until the next b_attn
                s0 = st * ST
                vraw = raws["v"]
                for j in range(ST // 128):
                    vtp = psm.tile([128, ST], QKV_DT, tag="misc", name="vtp")
                    nc.tensor.transpose(vtp[:, 0:128], vraw[:, j * 128:(j + 1) * 128], idn_sb)
                    nc.scalar.copy(vnat[:, (s0 // 128) + j, :], vtp[:, 0:128])
                # dummy exp: pulls the exp-set act-table reload off the next
                # attention segment's critical path (runs while ACT is idle
                # during the following c-loop)
                nc.scalar.activation(warm_sb[:], epsq_sb[:], AF.Exp)

            o_accs = {}

            def b_proj(i, dc_lo, dc_hi):
                q0 = i * QT
                if i not in o_accs:
                    o_accs[i] = sc_pool.tile([128, D // 128, QT], PR_DT, tag="osb",
                                             name="o_acc")
                o_acc = o_accs[i]
                last = (i == nqt - 1)
                for dc in range(dc_lo, dc_hi):
                    op_t = psa.tile([128, QT], F32, tag=f"qp{dc % 4}", name="op_t")
                    op = op_t[:]
                    for h in range(G):
                        nc.tensor.matmul(op, wp_sb[:, h, dc * 128:(dc + 1) * 128],
                                         yn[:, h, q0:q0 + QT],
                                         start=(h == 0), stop=(h == G - 1))
                    nc.vector.tensor_copy(o_acc[:, dc, :], op)
                    # out rides the ACT HWDGE queue so a gated descriptor
                    # never head-blocks the x loads on the SP queue
                    if last:
                        nc.scalar.dma_start(out=out[:, i, dc:dc + 1, :],
                                            in_=o_acc[:, dc:dc + 1, :])
                    elif dc in (3, 7, 11, 15):
                        nc.scalar.dma_start(out=out[:, i, dc - 3:dc + 1, :],
                                            in_=o_acc[:, dc - 3:dc + 1, :])

            def b_attn(i, spill=None):
                """Attention for q-tile i; spill=(j, lo, hi) emits proj
                matmuls of q-tile j into the h gaps so the ACT engine can
                drain its exp backlog while the PE does independent work."""
                q0 = i * QT
                nch_i = (QT // KC) * (i + 1)
                for h in range(G):
                    yp_t = psa.tile([128, QT], F32, tag=("kp" if h % 2 == 0 else "vp"),
                                    name=f"yp{h}")
                    sgp_t = psm.tile([128, QT], F32, tag="misc", name=f"sgp{h}")
                    yp = yp_t[:]
                    sgp = sgp_t[:]
                    dg0 = (QT // KC) * i
                    order = list(range(dg0, nch_i)) + list(range(0, dg0))
                    for gi, c in enumerate(order):
                        off = (c - dg0) * KC if c >= dg0 else 0
                        if gi == 0:
                            scp = psm.tile([128, QT], F32, tag="misc", name="scp0")
                        else:
                            scp = psa.tile([128, QT], F32, tag=f"qp{(gi - 1) % 4}",
                                           name="scp")
                        nc.tensor.matmul(scp[:, off:QT],
                                         khat[:, c * KC:(c + 1) * KC],
                                         qhat[:, h, q0 + off:q0 + QT],
                                         start=True, stop=True)
                        et = sb.tile([128, QT], AV_DT, tag="et", bufs=5)
                        nc.scalar.activation(et[:, off:QT], scp[:, off:QT], AF.Exp)
                        if c >= dg0:
                            # triangular mask on the diagonal 128-block; on
                            # gpsimd so it never queues behind the DVE drains
                            nc.gpsimd.tensor_mul(
                                et[:, off:off + KC], et[:, off:off + KC],
                                msk_sb)
                        nc.tensor.matmul(yp[:, off:QT], vnat[:, c, :],
                                         et[:, off:QT],
                                         start=(gi == 0),
                                         stop=(gi == nch_i - 1))
                        nc.tensor.matmul(sgp[:, off:QT], ons_sb,
                                         et[:, off:QT],
                                         start=(gi == 0),
                                         stop=(gi == nch_i - 1))
                    rs = sb.tile([128, QT], F32, tag="rs")
                    nc.vector.reciprocal_approx_fast(rs[:], sgp)
                    nc.vector.tensor_mul(yn[:, h, q0:q0 + QT], yp, rs[:])
                    if spill is not None and h < G - 1:
                        j, lo, hi = spill
                        w = (hi - lo) // 3
                        b_proj(j, lo + h * w, lo + (h + 1) * w if h < 2 else hi)

            # ------------- emission order -------------
            qkv = a_cloop(0)
            rr = a_drain(0, *qkv)
            a_chains_k(0, rr)
            a_chains_q(0, rr)
            for st in range(1, nst):
                qkv = a_cloop(st)
                rr = a_drain(st, *qkv)
                b_attn(st - 1, spill=(st - 2, 10, 16) if st >= 2 else None)
                a_chains_k(st, rr)
                if st == nst - 1:
                    # last s-tile: q chains feed b_attn(3) directly, so run
                    # them before the proj so they overlap its PE work
                    a_chains_q(st, rr, rope_dve=True)
                    b_proj(st - 1, 0, 10)
                else:
                    b_proj(st - 1, 0, 10)
                    a_chains_q(st, rr)
            b_attn(nqt - 1, spill=(nqt - 2, 10, 16))
            b_proj(nqt - 1, 0, 16)
    nc.compile()
    return nc


def _host_tables(s_len=S):
    inv_freq = 1.0 / (ROPE_BASE ** (np.arange(0, HD, 2, dtype=np.float64) / HD))
    t = np.arange(s_len, dtype=np.float64)
    freqs = np.outer(inv_freq, t)  # [64, S]
    c = np.cos(freqs)
    s_ = np.sin(freqs)
    cos2 = np.concatenate([c, c], axis=0).astype(np.float32)
    sin2 = np.concatenate([s_, -s_], axis=0).astype(np.float32)
    swp = np.zeros((128, 128), dtype=np.float32)
    swp[np.arange(64), np.arange(64) + 64] = 1.0
    swp[np.arange(64) + 64, np.arange(64)] = 1.0
    idn = np.eye(128, dtype=np.float32)
    ons = np.ones((128, 128), dtype=np.float32)
    p = np.arange(128)[:, None]
    f = np.arange(128)[None, :]
    msk = (p <= f).astype(np.float32)
    tabs = np.stack([swp, idn, ons, msk], axis=1)  # [128, 4, 128]
    return cos2, sin2, tabs


_NC_CACHE = {}


def _get_program(s_len=S):
    if s_len not in _NC_CACHE:
        _NC_CACHE[s_len] = _build_program(s_len)
    return _NC_CACHE[s_len]


def _np_dt(a, dt_):
    import ml_dtypes
    if dt_ == BF16:
        return np.ascontiguousarray(a).astype(ml_dtypes.bfloat16)
    return np.ascontiguousarray(np.asarray(a, dtype=np.float32))


def make_in_maps(x, Wq, Wk, Wv, Wproj, q_gain, s_len=S):
    x = np.asarray(x, dtype=np.float32)
    Wq = np.asarray(Wq, dtype=np.float32)
    Wk = np.asarray(Wk, dtype=np.float32)
    Wv = np.asarray(Wv, dtype=np.float32)
    Wproj = np.asarray(Wproj, dtype=np.float32)
    q_gain = np.asarray(q_gain, dtype=np.float32)
    cos2, sin2, tabs = _host_tables(s_len)
    nst = s_len // ST
    xas = []
    for b in range(B):
        xT = np.ascontiguousarray(x[b].T)                      # [D, S]
        # [st, g, p, j, m] with row = 128*(4g+j)+p, col = 512*st+m
        xa = xT.reshape(4, 4, 128, nst, ST).transpose(3, 0, 2, 1, 4)
        xas.append(_np_dt(xa, QKV_DT))
    in_maps = []
    for core in range(8):
        b, g = core // N_KV, core % N_KV
        wqT = np.ascontiguousarray(Wq[g * G * HD:(g + 1) * G * HD, :].T)
        wkT = np.ascontiguousarray(Wk[g * HD:(g + 1) * HD, :].T)
        wvT = np.ascontiguousarray(Wv[g * HD:(g + 1) * HD, :].T)
        wpT = np.ascontiguousarray(Wproj[:, g * G * HD:(g + 1) * G * HD].T)
        in_maps.append({
            "xa": xas[b],
            "wq": _np_dt(wqT.reshape(4, 4, 128, G * HD).transpose(0, 2, 1, 3), QKV_DT),
            "wk": _np_dt(wkT.reshape(4, 4, 128, HD).transpose(0, 2, 1, 3), QKV_DT),
            "wv": _np_dt(wvT.reshape(4, 4, 128, HD).transpose(0, 2, 1, 3), QKV_DT),
            "wp": _np_dt(wpT.reshape(G, 128, D).transpose(1, 0, 2), PR_DT),
            "cos2": _np_dt(cos2, SC_DT), "sin2": _np_dt(sin2, SC_DT),
            "tabs": _np_dt(tabs, BF16),
            "gains": np.broadcast_to(q_gain[g * G:(g + 1) * G][None, :],
                                     (128, G)).copy(),
        })
    return in_maps


def unshard(results):
    out = np.empty((B, S, D), dtype=np.float32)
    for b in range(B):
        acc = np.zeros((D, S), dtype=np.float32)
        for g in range(N_KV):
            o = np.asarray(results[4 * b + g]["out"], dtype=np.float32)
            acc += o.transpose(2, 0, 1, 3).reshape(D, S)
        out[b] = acc.T
    return out


def kernel(x, Wq, Wk, Wv, Wproj, q_gain):
    nc = _get_program(S)
    in_maps = make_in_maps(x, Wq, Wk, Wv, Wproj, q_gain, S)
    res = run_bass_kernel_spmd(nc, in_maps, list(range(8)))
    return unshard(res.results)

